# revision 3
# baseline (speedup 1.0000x reference)
"""Cross-attention layer (B=2, QL=CL=2048, E=1024, 16 heads x 64d) on 8 TRN2 cores.

Sharding: tensor-parallel over heads. Core c owns heads (2c, 2c+1): a 128-wide
feature slice of Wq/Wk/Wv columns and Wo rows. Each core computes a full-shape
partial of the output projection; the host sums the 8 partials and adds bo.

Schedule (per core): software-pipelined emission that keeps TensorE and the
scalar (ACT) engine gapless. Scores matmuls are K=64 row-group pairs (heads on
partitions 0-63 / 64-127) which the PE runs concurrently; exp paces the
attention loop at ~1.1us/c-tile; batch-1 projections are emitted as filler
matmuls inside batch-0's attention units; each unit's normalize + output
projection is deferred into the next unit's early slots so semaphore waits
never park at the head of the in-order tensor queue.

All DRAM tensors use host-prearranged tile-contiguous layouts so every DMA is
a single large contiguous descriptor.
"""

import numpy as np
import ml_dtypes

E = 1024          # embed dim
H = 16            # heads
D = 64            # head dim
B = 2
QL = CL = 2048
POS = B * QL      # 4096 flattened positions
NCORES = 8
P = 128           # per-core feature slice (2 heads x 64)
ET = E // 128     # 8 contraction e-tiles
QB = 512          # q-block (free dim of attention matmuls)
NQB = QL // QB    # 4 q-blocks per batch
CT = CL // 128    # 16 context tiles per batch
NPT = POS // 128  # 32 position tiles (V layout)
VW = 66           # per-head stride in V_sb blocks: 64 V cols + 1 ones + 1 pad
ICH = 2           # input DMA chunks per (e-tile, batch)
ICW = QL // ICH   # input chunk width (1024 positions)

BF16 = ml_dtypes.bfloat16

_CACHE = {}


def _build_nc():
    import concourse.bacc as bacc
    import concourse.mybir as mybir
    import concourse.tile as tile

    bf = mybir.dt.bfloat16
    f32 = mybir.dt.float32
    Exp = mybir.ActivationFunctionType.Exp
    mult = mybir.AluOpType.mult

    nc = bacc.Bacc(
        "TRN2",
        target_bir_lowering=False,
        debug=False,
        enable_asserts=False,
        num_devices=NCORES,
    )

    # inputs: tile-contiguous host layouts (1 descriptor per DMA)
    # qT/cT: [ET, B, ICH, 128, ICW]  (e-tile rows x position chunks)
    qT_d = nc.dram_tensor("qT", [ET * B * ICH, 128, ICW], bf, kind="ExternalInput").ap()
    cT_d = nc.dram_tensor("cT", [ET * B * ICH, 128, ICW], bf, kind="ExternalInput").ap()
    # weights: [128, ET*128] e-tile-major; wo: [128, E]
    wq_d = nc.dram_tensor("wq", [128, ET * P], bf, kind="ExternalInput").ap()
    wk_d = nc.dram_tensor("wk", [128, ET * P], bf, kind="ExternalInput").ap()
    wv_d = nc.dram_tensor("wv", [128, ET * P], bf, kind="ExternalInput").ap()
    wo_d = nc.dram_tensor("wo", [P, E], bf, kind="ExternalInput").ap()
    bq_d = nc.dram_tensor("bq", [P, 1], f32, kind="ExternalInput").ap()
    bk_d = nc.dram_tensor("bk", [P, 1], f32, kind="ExternalInput").ap()
    bv_d = nc.dram_tensor("bvt", [128, P], f32, kind="ExternalInput").ap()
    # output: tile-major [B*NQB*ET, 128, QB]; host reassembles to [B, QL, E]
    out_d = nc.dram_tensor("outt", [B * NQB * ET, 128, QB], bf, kind="ExternalOutput").ap()

    with tile.TileContext(nc) as tc:
        with (
            tc.tile_pool(name="const", bufs=1) as const,
            tc.tile_pool(name="inp", bufs=1) as inp,
            tc.tile_pool(name="proj", bufs=1) as proj,
            tc.tile_pool(name="egp", bufs=4) as egp,
            tc.tile_pool(name="zp", bufs=2) as zp,
            tc.tile_pool(name="obp", bufs=3) as obp,
            tc.tile_pool(name="ps_sg", bufs=2, space="PSUM") as ps_sg,
            tc.tile_pool(name="ps_att", bufs=2, space="PSUM") as ps_att,
            tc.tile_pool(name="ps_po", bufs=2, space="PSUM") as ps_po,
        ):
            # ---- input DMAs in priority order: ct-b0, qt-b0, ct-b1, qt-b1 --
            ct_sb = inp.tile([128, ET, POS], bf)
            qt_sb = inp.tile([128, ET, POS], bf)
            for b in range(B):
                for src_d, dst in ((cT_d, ct_sb), (qT_d, qt_sb)):
                    for t in range(ET):
                        for ch in range(ICH):
                            idx = (t * B + b) * ICH + ch
                            q0 = b * QL + ch * ICW
                            nc.sync.dma_start(
                                dst[:, t, q0 : q0 + ICW], src_d[idx : idx + 1]
                            )

            # ---- constants / weights ------------------------------------
            wq_sb = const.tile([128, ET, P], bf)
            wk_sb = const.tile([128, ET, P], bf)
            wv_sb = const.tile([128, ET, P], bf)
            nc.sync.dma_start(wq_sb[:], wq_d[:])
            nc.sync.dma_start(wk_sb[:], wk_d[:])
            nc.sync.dma_start(wv_sb[:], wv_d[:])
            wo_sb = const.tile([P, E], bf)
            nc.sync.dma_start(wo_sb[:], wo_d[:])
            bq_sb = const.tile([P, 1], f32)
            bk_sb = const.tile([P, 1], f32)
            bv_sb = const.tile([128, P], f32)
            nc.sync.dma_start(bq_sb[:], bq_d[:])
            nc.sync.dma_start(bk_sb[:], bk_d[:])
            nc.sync.dma_start(bv_sb[:], bv_d[:])
            ones65 = const.tile([65, 64], bf)
            nc.vector.memset(ones65[:], 1.0)

            # ---- persistent activations ---------------------------------
            kproj = proj.tile([P, POS], bf)   # K^T (2 heads on partitions)
            qproj = proj.tile([P, POS], bf)   # Q^T
            v_sb = proj.tile([128, NPT, 2 * VW], bf)  # V pos-major + ones col
            nc.vector.memset(v_sb[:], 1.0)
            an_sb = proj.tile([P, POS], bf)   # normalized attended^T

            # ---- emission helpers ---------------------------------------
            def qk_chunk(src_sb, w_sb, b_sb, dst, b, ch, nm):
                """One 512-wide projection chunk: 8 accumulating matmuls + bias evac."""
                q0 = b * QL + ch * QB
                ps = ps_po.tile([128, QB], f32, tag="po", name=f"pj{nm}{b}{ch}")
                for t in range(ET):
                    nc.tensor.matmul(
                        ps[:],
                        w_sb[:, t, :],
                        src_sb[:, t, q0 : q0 + QB],
                        start=(t == 0),
                        stop=(t == ET - 1),
                    )
                nc.vector.tensor_scalar_add(dst[:, q0 : q0 + QB], ps[:], b_sb[:])

            def v_pt(b, pti):
                """V projection for one 128-position tile (position-major)."""
                pt = b * CT + pti
                psv = ps_po.tile([128, 128], f32, tag="po", name=f"pv{pt}")
                for t in range(ET):
                    nc.tensor.matmul(
                        psv[:],
                        ct_sb[:, t, pt * 128 : (pt + 1) * 128],
                        wv_sb[:, t, :],
                        start=(t == 0),
                        stop=(t == ET - 1),
                    )
                nc.vector.tensor_add(v_sb[:, pt, 0:64], psv[:, 0:64], bv_sb[:, 0:64])
                nc.vector.tensor_add(
                    v_sb[:, pt, VW : VW + 64], psv[:, 64:128], bv_sb[:, 64:128]
                )

            # filler queue: list of closures, each emitting a small tensor-op
            # group (<=4 matmuls). Popped inside attention units to keep the
            # PE busy while the scalar engine paces the exp stream.
            fillers = []

            def pop_filler(n=1):
                for _ in range(n):
                    if fillers:
                        fillers.pop(0)()

            # ---- attention unit machinery -------------------------------
            # tail state from the previous unit, flushed into the next one
            tail = {}

            def emit_tail_zb():
                if not tail:
                    return
                for h in range(2):
                    zbp = ps_po.tile([64, QB], f32, tag="po", name=f"zb{tail['id']}{h}")
                    nc.tensor.matmul(
                        zbp[:],
                        ones65[64:65, :],
                        tail["attu"][h][64:65, :],
                        start=True,
                        stop=True,
                    )
                    ztr = zp.tile([64, QB], f32, tag="ztr", name=f"zt{tail['id']}{h}")
                    nc.vector.reciprocal_approx_fast(ztr[:], zbp[:])
                    if h == 0:
                        nc.vector.tensor_tensor(
                            an_sb[0:64, tail["q0"] : tail["q0"] + QB],
                            tail["attu"][h][0:64, :],
                            ztr[:],
                            op=mult,
                        )
                    else:
                        an1 = zp.tile([64, QB], bf, tag="an1", name=f"an{tail['id']}")
                        nc.vector.tensor_tensor(
                            an1[:], tail["attu"][h][0:64, :], ztr[:], op=mult
                        )
                        nc.sync.dma_start(
                            an_sb[64:128, tail["q0"] : tail["q0"] + QB], an1[:]
                        )

            def emit_tail_po(lo, hi):
                if not tail:
                    return
                b, qb = tail["b"], tail["qb"]
                q0 = tail["q0"]
                for eo in range(lo, hi):
                    po = ps_po.tile([128, QB], f32, tag="po", name=f"po{tail['id']}{eo}")
                    nc.tensor.matmul(
                        po[:],
                        wo_sb[:, eo * 128 : (eo + 1) * 128],
                        an_sb[:, q0 : q0 + QB],
                        start=True,
                        stop=True,
                    )
                    ob = obp.tile([128, QB], bf, tag="ob", name=f"ob{tail['id']}{eo}")
                    nc.vector.tensor_copy(ob[:], po[:])
                    oi = (b * NQB + qb) * ET + eo
                    nc.sync.dma_start(out_d[oi : oi + 1], ob[:])

            def attention_unit(b, qb):
                nonlocal tail
                uid = f"{b}{qb}"
                q0 = b * QL + qb * QB
                atts = [
                    ps_att.tile([65, QB], f32, tag="att", name=f"at{uid}{h}")
                    for h in range(2)
                ]
                egs = {}

                def scores_exp(ci):
                    c0 = b * CL + ci * 128
                    sg = ps_sg.tile([128, 2 * QB], f32, tag="sg", name=f"sg{uid}{ci}")
                    for h in range(2):
                        hp = h * 64
                        nc.tensor.matmul(
                            sg[:, h * QB : (h + 1) * QB],
                            kproj[hp : hp + 64, c0 : c0 + 128],
                            qproj[hp : hp + 64, q0 : q0 + QB],
                            start=True,
                            stop=True,
                        )
                    eg = egp.tile([128, 2 * QB], bf, tag="eg", name=f"eg{uid}{ci}")
                    nc.scalar.activation(eg[:], sg[:], Exp, scale=0.125)
                    egs[ci] = eg

                def attended(ci):
                    pt = b * CT + ci
                    eg = egs.pop(ci)
                    for h in range(2):
                        nc.tensor.matmul(
                            atts[h][:],
                            v_sb[:, pt, h * VW : h * VW + 65],
                            eg[:, h * QB : (h + 1) * QB],
                            start=(ci == 0),
                            stop=(ci == CT - 1),
                        )

                first = b == 0 and qb == 0
                for ci in range(CT):
                    scores_exp(ci)
                    if ci >= 2:
                        attended(ci - 2)
                    if ci == 2:
                        emit_tail_zb()
                    elif ci in (4, 5, 6, 7):
                        emit_tail_po(2 * (ci - 4), 2 * (ci - 3))
                    elif first:
                        if ci in (9, 11, 13):
                            pop_filler(1)
                    else:
                        pop_filler(1)
                attended(CT - 2)
                attended(CT - 1)

                # normalize DVE front-half now; tensor parts deferred to next unit
                attus = []
                for h in range(2):
                    attu = zp.tile([65, QB], bf, tag="attu", name=f"au{uid}{h}")
                    nc.vector.tensor_copy(attu[:], atts[h][:])
                    attus.append(attu)
                tail = {"id": uid, "b": b, "qb": qb, "q0": q0, "attu": attus}

            # ---- batch 0 projections (serial pre-phase) ------------------
            for ch in range(NQB):
                qk_chunk(ct_sb, wk_sb, bk_sb, kproj, 0, ch, "k")
            for pti in range(8):
                v_pt(0, pti)
            qk_chunk(qt_sb, wq_sb, bq_sb, qproj, 0, 0, "q")
            for pti in range(8, CT):
                v_pt(0, pti)

            # ---- fillers: rest of q-proj b0, then all b1 projections -----
            for ch in range(1, NQB):
                fillers.append(lambda ch=ch: qk_chunk(qt_sb, wq_sb, bq_sb, qproj, 0, ch, "q"))
            for ch in range(NQB):
                fillers.append(lambda ch=ch: qk_chunk(ct_sb, wk_sb, bk_sb, kproj, 1, ch, "k"))
            for pti in range(CT):
                fillers.append(lambda pti=pti: v_pt(1, pti))
            for ch in range(NQB):
                fillers.append(lambda ch=ch: qk_chunk(qt_sb, wq_sb, bq_sb, qproj, 1, ch, "q"))

            # ---- attention over all units --------------------------------
            for b in range(B):
                for qb in range(NQB):
                    attention_unit(b, qb)

            # flush remaining fillers (shouldn't be any) and final tail
            pop_filler(len(fillers))
            emit_tail_zb()
            emit_tail_po(0, ET)

    nc.compile()
    return nc


def get_nc():
    if "nc" not in _CACHE:
        _CACHE["nc"] = _build_nc()
    return _CACHE["nc"]


def make_in_maps(query, context, Wq, bq, Wk, bk, Wv, bv, Wo, bo):
    # qT/cT tile-contiguous: [ET, B, ICH, 128, ICW]
    def pack_acts(x):
        xt = np.asarray(x, np.float32).reshape(POS, E).T.astype(BF16)  # [E, POS]
        xt = xt.reshape(ET, 128, B, ICH, ICW)  # split rows into e-tiles, cols into chunks
        xt = xt.transpose(0, 2, 3, 1, 4)  # [ET, B, ICH, 128, ICW]
        return np.ascontiguousarray(xt).reshape(ET * B * ICH, 128, ICW)

    qT = pack_acts(query)
    cT = pack_acts(context)

    def pack_w(Wslice):  # [E, P] -> [128, ET*P] e-tile-major
        w = np.asarray(Wslice, np.float32).reshape(ET, 128, P).transpose(1, 0, 2)
        return np.ascontiguousarray(w).astype(BF16).reshape(128, ET * P)

    in_maps = []
    for c in range(NCORES):
        F = slice(P * c, P * (c + 1))
        in_maps.append(
            {
                "qT": qT,
                "cT": cT,
                "wq": pack_w(Wq[:, F]),
                "wk": pack_w(Wk[:, F]),
                "wv": pack_w(Wv[:, F]),
                "wo": np.ascontiguousarray(Wo[F, :]).astype(BF16),
                "bq": np.ascontiguousarray(bq[F]).reshape(P, 1).astype(np.float32),
                "bk": np.ascontiguousarray(bk[F]).reshape(P, 1).astype(np.float32),
                "bvt": np.ascontiguousarray(
                    np.broadcast_to(bv[F], (128, P))
                ).astype(np.float32),
            }
        )
    return in_maps


def assemble_output(partials, bo):
    # partials: per-core [B*NQB*ET, 128, QB] bf16 tile lists
    total = np.zeros((B * NQB * ET, 128, QB), np.float32)
    for p in partials:
        total += p
    # [B, NQB, ET, 128, QB] -> [B, NQB*QB(q), ET*128(e)]
    t = total.reshape(B, NQB, ET, 128, QB).transpose(0, 1, 4, 2, 3)
    out = t.reshape(B, QL, E) + np.asarray(bo, np.float32)
    return out.astype(np.float32)


def kernel(query, context, Wq, bq, Wk, bk, Wv, bv, Wo, bo):
    from concourse import bass_utils

    nc = get_nc()
    in_maps = make_in_maps(query, context, Wq, bq, Wk, bk, Wv, bv, Wo, bo)
    res = bass_utils.run_bass_kernel_spmd(nc, in_maps, core_ids=list(range(NCORES)))
    partials = [res.results[c]["outt"] for c in range(NCORES)]
    return assemble_output(partials, bo)


# revision 4
# speedup vs baseline: 1.3195x; 1.3195x over previous
"""Cross-attention layer (B=2, QL=CL=2048, E=1024, 16 heads x 64d) on 8 TRN2 cores.

Sharding: tensor-parallel over heads. Core c owns heads (2c, 2c+1): a 128-wide
feature slice of Wq/Wk/Wv columns and Wo rows. Each core computes a full-shape
partial of the output projection; the host sums the 8 partials and adds bo.

Schedule (per core): software-pipelined emission that keeps TensorE and the
scalar (ACT) engine gapless. Scores matmuls are K=64 row-group pairs (heads on
partitions 0-63 / 64-127) which the PE runs concurrently; exp paces the
attention loop at ~1.1us/c-tile; batch-1 projections are emitted as half-chain
fillers inside batch-0's attention units; each unit's normalize + output
projection is deferred into the next unit's early slots so semaphore waits
never park at the head of the in-order tensor queue. DMA issue is serial on
the sync engine (~0.67us per dma_start), so DMA instruction count is kept low
and ordered: weights first, then ct-b0, qt-b0, ct-b1, qt-b1.
"""

import numpy as np
import ml_dtypes

E = 1024          # embed dim
H = 16            # heads
D = 64            # head dim
B = 2
QL = CL = 2048
POS = B * QL      # 4096 flattened positions
NCORES = 8
P = 128           # per-core feature slice (2 heads x 64)
ET = E // 128     # 8 contraction e-tiles
QB = 512          # q-block (free dim of attention matmuls)
NQB = QL // QB    # 4 q-blocks per batch
CT = CL // 128    # 16 context tiles per batch
NPT = POS // 128  # 32 position tiles (V layout)
VW = 66           # per-head stride in V_sb blocks: 64 V cols + 1 ones + 1 pad
OBW = 4 * QB      # output staging width (4 e-chunks per DMA, 4KB segments)

BF16 = ml_dtypes.bfloat16

_CACHE = {}


def _build_nc():
    import concourse.bacc as bacc
    import concourse.mybir as mybir
    import concourse.tile as tile

    bf = mybir.dt.bfloat16
    f32 = mybir.dt.float32
    Exp = mybir.ActivationFunctionType.Exp
    mult = mybir.AluOpType.mult

    nc = bacc.Bacc(
        "TRN2",
        target_bir_lowering=False,
        debug=False,
        enable_asserts=False,
        num_devices=NCORES,
    )

    # inputs: tile-contiguous host layouts: [ET*B, 128, QL] (e-tile x batch)
    qT_d = nc.dram_tensor("qT", [ET * B, 128, QL], bf, kind="ExternalInput").ap()
    cT_d = nc.dram_tensor("cT", [ET * B, 128, QL], bf, kind="ExternalInput").ap()
    wq_d = nc.dram_tensor("wq", [128, ET * P], bf, kind="ExternalInput").ap()
    wk_d = nc.dram_tensor("wk", [128, ET * P], bf, kind="ExternalInput").ap()
    wv_d = nc.dram_tensor("wv", [128, ET * P], bf, kind="ExternalInput").ap()
    wo_d = nc.dram_tensor("wo", [P, E], bf, kind="ExternalInput").ap()
    bq_d = nc.dram_tensor("bq", [P, 1], f32, kind="ExternalInput").ap()
    bk_d = nc.dram_tensor("bk", [P, 1], f32, kind="ExternalInput").ap()
    bv_d = nc.dram_tensor("bvt", [128, P], f32, kind="ExternalInput").ap()
    # output: [B*NQB*2, 128, OBW]; host reassembles to [B, QL, E]
    out_d = nc.dram_tensor("outt", [B * NQB * 2, 128, OBW], bf, kind="ExternalOutput").ap()

    with tile.TileContext(nc) as tc:
        with (
            tc.tile_pool(name="const", bufs=1) as const,
            tc.tile_pool(name="inp", bufs=1) as inp,
            tc.tile_pool(name="proj", bufs=1) as proj,
            tc.tile_pool(name="egp", bufs=4) as egp,
            tc.tile_pool(name="zp", bufs=2) as zp,
            tc.tile_pool(name="obp", bufs=2) as obp,
            tc.tile_pool(name="ps_sg", bufs=2, space="PSUM") as ps_sg,
            tc.tile_pool(name="ps_att", bufs=2, space="PSUM") as ps_att,
            tc.tile_pool(name="ps_po", bufs=2, space="PSUM") as ps_po,
        ):
            # ---- SBUF tiles ---------------------------------------------
            ct_sb = inp.tile([128, ET, POS], bf)
            qt_sb = inp.tile([128, ET, POS], bf)
            wq_sb = const.tile([128, ET, P], bf)
            wk_sb = const.tile([128, ET, P], bf)
            wv_sb = const.tile([128, ET, P], bf)
            wo_sb = const.tile([P, E], bf)
            bq_sb = const.tile([P, 1], f32)
            bk_sb = const.tile([P, 1], f32)
            bv_sb = const.tile([128, P], f32)
            ones65 = const.tile([65, 64], bf)
            nc.vector.memset(ones65[:], 1.0)

            # ---- DMA in priority order ----------------------------------
            def in_batch(src_d, dst, b):
                for t in range(ET):
                    nc.sync.dma_start(
                        dst[:, t, b * QL : (b + 1) * QL], src_d[t * B + b : t * B + b + 1]
                    )

            nc.sync.dma_start(wk_sb[:], wk_d[:])
            nc.sync.dma_start(wv_sb[:], wv_d[:])
            in_batch(cT_d, ct_sb, 0)
            nc.sync.dma_start(wq_sb[:], wq_d[:])
            nc.sync.dma_start(bk_sb[:], bk_d[:])
            nc.sync.dma_start(bq_sb[:], bq_d[:])
            nc.sync.dma_start(bv_sb[:], bv_d[:])
            in_batch(qT_d, qt_sb, 0)
            nc.sync.dma_start(wo_sb[:], wo_d[:])
            in_batch(cT_d, ct_sb, 1)
            in_batch(qT_d, qt_sb, 1)

            # ---- persistent activations ---------------------------------
            kproj = proj.tile([P, POS], bf)   # K^T (2 heads on partitions)
            qproj = proj.tile([P, POS], bf)   # Q^T
            v_sb = proj.tile([128, NPT, 2 * VW], bf)  # V pos-major + ones col
            nc.vector.memset(v_sb[:], 1.0)
            an_sb = proj.tile([P, POS], bf)   # normalized attended^T

            # ---- emission helpers ---------------------------------------
            def qk_chunk_parts(src_sb, w_sb, b_sb, dst, b, ch, nm):
                """512-wide projection chunk as two 4-matmul filler parts."""
                q0 = b * QL + ch * QB
                st = {}

                def part1():
                    st["ps"] = ps_po.tile(
                        [128, QB], f32, tag="po", name=f"pj{nm}{b}{ch}"
                    )
                    for t in range(4):
                        nc.tensor.matmul(
                            st["ps"][:],
                            w_sb[:, t, :],
                            src_sb[:, t, q0 : q0 + QB],
                            start=(t == 0),
                            stop=False,
                        )

                def part2():
                    for t in range(4, ET):
                        nc.tensor.matmul(
                            st["ps"][:],
                            w_sb[:, t, :],
                            src_sb[:, t, q0 : q0 + QB],
                            start=False,
                            stop=(t == ET - 1),
                        )
                    nc.vector.tensor_scalar_add(dst[:, q0 : q0 + QB], st["ps"][:], b_sb[:])

                return [part1, part2]

            def qk_chunk(src_sb, w_sb, b_sb, dst, b, ch, nm):
                for p in qk_chunk_parts(src_sb, w_sb, b_sb, dst, b, ch, nm):
                    p()

            def v_pt_parts(b, pti):
                """V projection for one 128-position tile, two filler parts."""
                pt = b * CT + pti
                st = {}

                def part1():
                    st["ps"] = ps_po.tile([128, 128], f32, tag="po", name=f"pv{pt}")
                    for t in range(4):
                        nc.tensor.matmul(
                            st["ps"][:],
                            ct_sb[:, t, pt * 128 : (pt + 1) * 128],
                            wv_sb[:, t, :],
                            start=(t == 0),
                            stop=False,
                        )

                def part2():
                    for t in range(4, ET):
                        nc.tensor.matmul(
                            st["ps"][:],
                            ct_sb[:, t, pt * 128 : (pt + 1) * 128],
                            wv_sb[:, t, :],
                            start=False,
                            stop=(t == ET - 1),
                        )
                    psv = st["ps"]
                    nc.vector.tensor_add(v_sb[:, pt, 0:64], psv[:, 0:64], bv_sb[:, 0:64])
                    nc.vector.tensor_add(
                        v_sb[:, pt, VW : VW + 64], psv[:, 64:128], bv_sb[:, 64:128]
                    )

                return [part1, part2]

            def v_pt(b, pti):
                for p in v_pt_parts(b, pti):
                    p()

            fillers = []

            def pop_filler(n=1):
                for _ in range(n):
                    if fillers:
                        fillers.pop(0)()

            # ---- attention unit machinery -------------------------------
            tail = {}

            def emit_tail_zb():
                if not tail:
                    return
                for h in range(2):
                    zbp = ps_po.tile([64, QB], f32, tag="po", name=f"zb{tail['id']}{h}")
                    nc.tensor.matmul(
                        zbp[:],
                        ones65[64:65, :],
                        tail["attu"][h][64:65, :],
                        start=True,
                        stop=True,
                    )
                    ztr = zp.tile([64, QB], f32, tag="ztr", name=f"zt{tail['id']}{h}")
                    nc.vector.reciprocal_approx_fast(ztr[:], zbp[:])
                    if h == 0:
                        nc.vector.tensor_tensor(
                            an_sb[0:64, tail["q0"] : tail["q0"] + QB],
                            tail["attu"][h][0:64, :],
                            ztr[:],
                            op=mult,
                        )
                    else:
                        an1 = zp.tile([64, QB], bf, tag="an1", name=f"an{tail['id']}")
                        nc.vector.tensor_tensor(
                            an1[:], tail["attu"][h][0:64, :], ztr[:], op=mult
                        )
                        nc.sync.dma_start(
                            an_sb[64:128, tail["q0"] : tail["q0"] + QB], an1[:]
                        )

            def emit_tail_po(g):
                """Output projection for e-chunk group g (4 chunks) + staged DMA."""
                if not tail:
                    return
                b, qb = tail["b"], tail["qb"]
                q0 = tail["q0"]
                ob = obp.tile([128, OBW], bf, tag="ob", name=f"ob{tail['id']}{g}")
                for ei in range(4):
                    eo = g * 4 + ei
                    po = ps_po.tile([128, QB], f32, tag="po", name=f"po{tail['id']}{eo}")
                    nc.tensor.matmul(
                        po[:],
                        wo_sb[:, eo * 128 : (eo + 1) * 128],
                        an_sb[:, q0 : q0 + QB],
                        start=True,
                        stop=True,
                    )
                    nc.vector.tensor_copy(ob[:, ei * QB : (ei + 1) * QB], po[:])
                oi = (b * NQB + qb) * 2 + g
                nc.sync.dma_start(out_d[oi : oi + 1], ob[:])

            def attention_unit(b, qb):
                nonlocal tail
                uid = f"{b}{qb}"
                q0 = b * QL + qb * QB
                atts = [
                    ps_att.tile([65, QB], f32, tag="att", name=f"at{uid}{h}")
                    for h in range(2)
                ]
                egs = {}

                def scores_exp(ci):
                    c0 = b * CL + ci * 128
                    sg = ps_sg.tile([128, 2 * QB], f32, tag="sg", name=f"sg{uid}{ci}")
                    for h in range(2):
                        hp = h * 64
                        nc.tensor.matmul(
                            sg[:, h * QB : (h + 1) * QB],
                            kproj[hp : hp + 64, c0 : c0 + 128],
                            qproj[hp : hp + 64, q0 : q0 + QB],
                            start=True,
                            stop=True,
                        )
                    eg = egp.tile([128, 2 * QB], bf, tag="eg", name=f"eg{uid}{ci}")
                    nc.scalar.activation(eg[:], sg[:], Exp, scale=0.125)
                    egs[ci] = eg

                def attended(ci):
                    pt = b * CT + ci
                    eg = egs.pop(ci)
                    for h in range(2):
                        nc.tensor.matmul(
                            atts[h][:],
                            v_sb[:, pt, h * VW : h * VW + 65],
                            eg[:, h * QB : (h + 1) * QB],
                            start=(ci == 0),
                            stop=(ci == CT - 1),
                        )

                first = b == 0 and qb == 0
                for ci in range(CT):
                    scores_exp(ci)
                    if ci >= 2:
                        attended(ci - 2)
                    if first:
                        if ci in (11, 13, 15):
                            pop_filler(1)
                        continue
                    if ci == 2:
                        emit_tail_zb()
                    elif ci in (4, 6):
                        emit_tail_po((ci - 4) // 2)
                    elif ci in (0, 1, 3):
                        pop_filler(1)
                    elif ci >= 8:
                        pop_filler(2)
                attended(CT - 2)
                attended(CT - 1)

                attus = []
                for h in range(2):
                    attu = zp.tile([65, QB], bf, tag="attu", name=f"au{uid}{h}")
                    nc.vector.tensor_copy(attu[:], atts[h][:])
                    attus.append(attu)
                tail = {"id": uid, "b": b, "qb": qb, "q0": q0, "attu": attus}

            # ---- batch 0 projections (serial pre-phase) ------------------
            for ch in range(NQB):
                qk_chunk(ct_sb, wk_sb, bk_sb, kproj, 0, ch, "k")
            for pti in range(8):
                v_pt(0, pti)
            qk_chunk(qt_sb, wq_sb, bq_sb, qproj, 0, 0, "q")
            for pti in range(8, CT):
                v_pt(0, pti)

            # ---- fillers (order respects downstream deadlines) -----------
            for ch in range(1, NQB):
                fillers.extend(qk_chunk_parts(qt_sb, wq_sb, bq_sb, qproj, 0, ch, "q"))
            for ch in range(NQB):
                fillers.extend(qk_chunk_parts(ct_sb, wk_sb, bk_sb, kproj, 1, ch, "k"))
            for pti in range(CT):
                fillers.extend(v_pt_parts(1, pti))
            for ch in range(NQB):
                fillers.extend(qk_chunk_parts(qt_sb, wq_sb, bq_sb, qproj, 1, ch, "q"))

            # ---- attention over all units --------------------------------
            for b in range(B):
                for qb in range(NQB):
                    attention_unit(b, qb)

            pop_filler(len(fillers))
            emit_tail_zb()
            emit_tail_po(0)
            emit_tail_po(1)

    nc.compile()
    return nc


def get_nc():
    if "nc" not in _CACHE:
        _CACHE["nc"] = _build_nc()
    return _CACHE["nc"]


def make_in_maps(query, context, Wq, bq, Wk, bk, Wv, bv, Wo, bo):
    # qT/cT tile-contiguous: [ET, B, 128, QL]
    def pack_acts(x):
        xt = np.asarray(x, np.float32).reshape(POS, E).T.astype(BF16)  # [E, POS]
        xt = xt.reshape(ET, 128, B, QL).transpose(0, 2, 1, 3)  # [ET, B, 128, QL]
        return np.ascontiguousarray(xt).reshape(ET * B, 128, QL)

    qT = pack_acts(query)
    cT = pack_acts(context)

    def pack_w(Wslice):  # [E, P] -> [128, ET*P] e-tile-major
        w = np.asarray(Wslice, np.float32).reshape(ET, 128, P).transpose(1, 0, 2)
        return np.ascontiguousarray(w).astype(BF16).reshape(128, ET * P)

    in_maps = []
    for c in range(NCORES):
        F = slice(P * c, P * (c + 1))
        in_maps.append(
            {
                "qT": qT,
                "cT": cT,
                "wq": pack_w(Wq[:, F]),
                "wk": pack_w(Wk[:, F]),
                "wv": pack_w(Wv[:, F]),
                "wo": np.ascontiguousarray(Wo[F, :]).astype(BF16),
                "bq": np.ascontiguousarray(bq[F]).reshape(P, 1).astype(np.float32),
                "bk": np.ascontiguousarray(bk[F]).reshape(P, 1).astype(np.float32),
                "bvt": np.ascontiguousarray(
                    np.broadcast_to(bv[F], (128, P))
                ).astype(np.float32),
            }
        )
    return in_maps


def assemble_output(partials, bo):
    # partials: per-core [B*NQB*2, 128, OBW] tiles; value[p, ei*QB+q] of tile
    # (b, qb, g) = outT[(g*4+ei)*128 + p, b*QL + qb*QB + q]
    total = np.zeros((B * NQB * 2, 128, OBW), np.float32)
    for p in partials:
        total += p
    t = total.reshape(B, NQB, 2, 128, 4, QB).transpose(0, 1, 5, 2, 4, 3)
    out = t.reshape(B, QL, E) + np.asarray(bo, np.float32)
    return out.astype(np.float32)


def kernel(query, context, Wq, bq, Wk, bk, Wv, bv, Wo, bo):
    from concourse import bass_utils

    nc = get_nc()
    in_maps = make_in_maps(query, context, Wq, bq, Wk, bk, Wv, bv, Wo, bo)
    res = bass_utils.run_bass_kernel_spmd(nc, in_maps, core_ids=list(range(NCORES)))
    partials = [res.results[c]["outt"] for c in range(NCORES)]
    return assemble_output(partials, bo)


# revision 7
# speedup vs baseline: 1.3286x; 1.0069x over previous
"""Cross-attention layer (B=2, QL=CL=2048, E=1024, 16 heads x 64d) on 8 TRN2 cores.

Sharding: tensor-parallel over heads. Core c owns heads (2c, 2c+1): a 128-wide
feature slice of Wq/Wk/Wv columns and Wo rows. Each core computes a full-shape
partial of the output projection; the host sums the 8 partials and adds bo.

Schedule (per core): software-pipelined emission that keeps TensorE and the
scalar (ACT) engine gapless. Scores matmuls are K=64 row-group pairs (heads on
partitions 0-63 / 64-127) which the PE runs concurrently; exp paces the
attention loop at ~1.1us/c-tile; batch-1 projections are emitted as half-chain
fillers inside batch-0's attention units; each unit's normalize + output
projection is deferred into the next unit's early slots so semaphore waits
never park at the head of the in-order tensor queue. DMA issue is serial on
the sync engine (~0.67us per dma_start), so DMA instruction count is kept low
and ordered: weights first, then ct-b0, qt-b0, ct-b1, qt-b1.
"""

import numpy as np
import ml_dtypes

E = 1024          # embed dim
H = 16            # heads
D = 64            # head dim
B = 2
QL = CL = 2048
POS = B * QL      # 4096 flattened positions
NCORES = 8
P = 128           # per-core feature slice (2 heads x 64)
ET = E // 128     # 8 contraction e-tiles
QB = 512          # q-block (free dim of attention matmuls)
NQB = QL // QB    # 4 q-blocks per batch
CT = CL // 128    # 16 context tiles per batch
NPT = POS // 128  # 32 position tiles (V layout)
VW = 66           # per-head stride in V_sb blocks: 64 V cols + 1 ones + 1 pad
OBW = 4 * QB      # output staging width (4 e-chunks per DMA, 4KB segments)

BF16 = ml_dtypes.bfloat16

_CACHE = {}


def _build_nc():
    import concourse.bacc as bacc
    import concourse.mybir as mybir
    import concourse.tile as tile

    bf = mybir.dt.bfloat16
    f32 = mybir.dt.float32
    Exp = mybir.ActivationFunctionType.Exp
    mult = mybir.AluOpType.mult

    nc = bacc.Bacc(
        "TRN2",
        target_bir_lowering=False,
        debug=False,
        enable_asserts=False,
        num_devices=NCORES,
    )

    # inputs: tile-contiguous host layouts: [ET*B, 128, QL] (e-tile x batch)
    qT_d = nc.dram_tensor("qT", [ET * B, 128, QL], bf, kind="ExternalInput").ap()
    cT_d = nc.dram_tensor("cT", [ET * B, 128, QL], bf, kind="ExternalInput").ap()
    wq_d = nc.dram_tensor("wq", [128, ET * P], bf, kind="ExternalInput").ap()
    wk_d = nc.dram_tensor("wk", [128, ET * P], bf, kind="ExternalInput").ap()
    wv_d = nc.dram_tensor("wv", [128, ET * P], bf, kind="ExternalInput").ap()
    wo_d = nc.dram_tensor("wo", [P, E], bf, kind="ExternalInput").ap()
    bq_d = nc.dram_tensor("bq", [P, 1], f32, kind="ExternalInput").ap()
    bk_d = nc.dram_tensor("bk", [P, 1], f32, kind="ExternalInput").ap()
    bv_d = nc.dram_tensor("bvt", [128, P], f32, kind="ExternalInput").ap()
    # output: [B*NQB*2, 128, OBW]; host reassembles to [B, QL, E]
    out_d = nc.dram_tensor("outt", [B * NQB * 2, 128, OBW], bf, kind="ExternalOutput").ap()

    with tile.TileContext(nc) as tc:
        with (
            tc.tile_pool(name="const", bufs=1) as const,
            tc.tile_pool(name="inp", bufs=1) as inp,
            tc.tile_pool(name="proj", bufs=1) as proj,
            tc.tile_pool(name="egp", bufs=4) as egp,
            tc.tile_pool(name="zp", bufs=2) as zp,
            tc.tile_pool(name="obp", bufs=2) as obp,
            tc.tile_pool(name="ps_sg", bufs=2, space="PSUM") as ps_sg,
            tc.tile_pool(name="ps_att", bufs=2, space="PSUM") as ps_att,
            tc.tile_pool(name="ps_po", bufs=2, space="PSUM") as ps_po,
        ):
            # ---- SBUF tiles ---------------------------------------------
            ct_sb = inp.tile([128, ET, POS], bf)
            qt_sb = inp.tile([128, ET, POS], bf)
            wq_sb = const.tile([128, ET, P], bf)
            wk_sb = const.tile([128, ET, P], bf)
            wv_sb = const.tile([128, ET, P], bf)
            wo_sb = const.tile([P, E], bf)
            bq_sb = const.tile([P, 1], f32)
            bk_sb = const.tile([P, 1], f32)
            bv_sb = const.tile([128, P], f32)
            ones65 = const.tile([65, 64], bf)
            nc.vector.memset(ones65[:], 1.0)

            # ---- DMA in priority order ----------------------------------
            def in_batch(src_d, dst, b):
                for t in range(ET):
                    nc.sync.dma_start(
                        dst[:, t, b * QL : (b + 1) * QL], src_d[t * B + b : t * B + b + 1]
                    )

            nc.sync.dma_start(wk_sb[:], wk_d[:])
            nc.sync.dma_start(wv_sb[:], wv_d[:])
            in_batch(cT_d, ct_sb, 0)
            nc.sync.dma_start(wq_sb[:], wq_d[:])
            nc.sync.dma_start(bk_sb[:], bk_d[:])
            nc.sync.dma_start(bq_sb[:], bq_d[:])
            nc.sync.dma_start(bv_sb[:], bv_d[:])
            in_batch(qT_d, qt_sb, 0)
            nc.sync.dma_start(wo_sb[:], wo_d[:])
            in_batch(cT_d, ct_sb, 1)
            in_batch(qT_d, qt_sb, 1)

            # ---- persistent activations ---------------------------------
            kproj = proj.tile([P, POS], bf)   # K^T (2 heads on partitions)
            qproj = proj.tile([P, POS], bf)   # Q^T
            v_sb = proj.tile([128, NPT, 2 * VW], bf)  # V pos-major + ones col
            nc.vector.memset(v_sb[:], 1.0)
            an_sb = proj.tile([P, POS], bf)   # normalized attended^T

            # ---- emission helpers ---------------------------------------
            def qk_chunk_parts(src_sb, w_sb, b_sb, dst, b, ch, nm):
                """512-wide projection chunk as two 4-matmul filler parts."""
                q0 = b * QL + ch * QB
                st = {}

                def part1():
                    st["ps"] = ps_po.tile(
                        [128, QB], f32, tag="po", name=f"pj{nm}{b}{ch}"
                    )
                    for t in range(4):
                        nc.tensor.matmul(
                            st["ps"][:],
                            w_sb[:, t, :],
                            src_sb[:, t, q0 : q0 + QB],
                            start=(t == 0),
                            stop=False,
                        )

                def part2():
                    for t in range(4, ET):
                        nc.tensor.matmul(
                            st["ps"][:],
                            w_sb[:, t, :],
                            src_sb[:, t, q0 : q0 + QB],
                            start=False,
                            stop=(t == ET - 1),
                        )
                    nc.vector.tensor_scalar_add(dst[:, q0 : q0 + QB], st["ps"][:], b_sb[:])

                return [part1, part2]

            def qk_chunk(src_sb, w_sb, b_sb, dst, b, ch, nm):
                for p in qk_chunk_parts(src_sb, w_sb, b_sb, dst, b, ch, nm):
                    p()

            def v_pt_parts(b, pti):
                """V projection for one 128-position tile, two filler parts."""
                pt = b * CT + pti
                st = {}

                def part1():
                    st["ps"] = ps_po.tile([128, 128], f32, tag="po", name=f"pv{pt}")
                    for t in range(4):
                        nc.tensor.matmul(
                            st["ps"][:],
                            ct_sb[:, t, pt * 128 : (pt + 1) * 128],
                            wv_sb[:, t, :],
                            start=(t == 0),
                            stop=False,
                        )

                def part2():
                    for t in range(4, ET):
                        nc.tensor.matmul(
                            st["ps"][:],
                            ct_sb[:, t, pt * 128 : (pt + 1) * 128],
                            wv_sb[:, t, :],
                            start=False,
                            stop=(t == ET - 1),
                        )
                    psv = st["ps"]
                    nc.vector.tensor_add(v_sb[:, pt, 0:64], psv[:, 0:64], bv_sb[:, 0:64])
                    nc.vector.tensor_add(
                        v_sb[:, pt, VW : VW + 64], psv[:, 64:128], bv_sb[:, 64:128]
                    )

                return [part1, part2]

            def v_pt(b, pti):
                for p in v_pt_parts(b, pti):
                    p()

            fillers = []

            def pop_filler(n=1):
                for _ in range(n):
                    if fillers:
                        fillers.pop(0)()

            # ---- attention unit machinery -------------------------------
            tail = {}

            def emit_tail_zb():
                if not tail:
                    return
                for h in range(2):
                    zbp = ps_po.tile([64, QB], f32, tag="po", name=f"zb{tail['id']}{h}")
                    nc.tensor.matmul(
                        zbp[:],
                        ones65[64:65, :],
                        tail["attu"][h][64:65, :],
                        start=True,
                        stop=True,
                    )
                    ztr = zp.tile([64, QB], f32, tag="ztr", name=f"zt{tail['id']}{h}")
                    nc.vector.reciprocal_approx_fast(ztr[:], zbp[:])
                    if h == 0:
                        nc.vector.tensor_tensor(
                            an_sb[0:64, tail["q0"] : tail["q0"] + QB],
                            tail["attu"][h][0:64, :],
                            ztr[:],
                            op=mult,
                        )
                    else:
                        an1 = zp.tile([64, QB], bf, tag="an1", name=f"an{tail['id']}")
                        nc.vector.tensor_tensor(
                            an1[:], tail["attu"][h][0:64, :], ztr[:], op=mult
                        )
                        nc.sync.dma_start(
                            an_sb[64:128, tail["q0"] : tail["q0"] + QB], an1[:]
                        )

            def emit_tail_po(g):
                """Output projection for e-chunk group g (4 chunks) + staged DMA."""
                if not tail:
                    return
                b, qb = tail["b"], tail["qb"]
                q0 = tail["q0"]
                ob = obp.tile([128, OBW], bf, tag="ob", name=f"ob{tail['id']}{g}")
                for ei in range(4):
                    eo = g * 4 + ei
                    po = ps_po.tile([128, QB], f32, tag="po", name=f"po{tail['id']}{eo}")
                    nc.tensor.matmul(
                        po[:],
                        wo_sb[:, eo * 128 : (eo + 1) * 128],
                        an_sb[:, q0 : q0 + QB],
                        start=True,
                        stop=True,
                    )
                    nc.vector.tensor_copy(ob[:, ei * QB : (ei + 1) * QB], po[:])
                oi = (b * NQB + qb) * 2 + g
                nc.sync.dma_start(out_d[oi : oi + 1], ob[:])

            def emit_tail_attu():
                # finish prev unit: last two attended pairs, then PSUM->SBUF
                if not tail:
                    return
                tail["attended"](CT - 2)
                tail["attended"](CT - 1)
                attus = []
                for h in range(2):
                    attu = zp.tile([65, QB], bf, tag="attu", name=f"au{tail['id']}{h}")
                    nc.vector.tensor_copy(attu[:], tail["atts"][h][:])
                    attus.append(attu)
                tail["attu"] = attus

            def attention_unit(b, qb):
                nonlocal tail
                uid = f"{b}{qb}"
                q0 = b * QL + qb * QB
                atts = [
                    ps_att.tile([65, QB], f32, tag="att", name=f"at{uid}{h}")
                    for h in range(2)
                ]
                egs = {}

                def scores_exp(ci):
                    c0 = b * CL + ci * 128
                    sg = ps_sg.tile([128, 2 * QB], f32, tag="sg", name=f"sg{uid}{ci}")
                    for h in range(2):
                        hp = h * 64
                        nc.tensor.matmul(
                            sg[:, h * QB : (h + 1) * QB],
                            kproj[hp : hp + 64, c0 : c0 + 128],
                            qproj[hp : hp + 64, q0 : q0 + QB],
                            start=True,
                            stop=True,
                        )
                    eg = egp.tile([128, 2 * QB], bf, tag="eg", name=f"eg{uid}{ci}")
                    nc.scalar.activation(eg[:], sg[:], Exp, scale=0.125)
                    egs[ci] = eg

                def attended(ci):
                    pt = b * CT + ci
                    eg = egs.pop(ci)
                    for h in range(2):
                        nc.tensor.matmul(
                            atts[h][:],
                            v_sb[:, pt, h * VW : h * VW + 65],
                            eg[:, h * QB : (h + 1) * QB],
                            start=(ci == 0),
                            stop=(ci == CT - 1),
                        )

                first = b == 0 and qb == 0
                for ci in range(CT):
                    scores_exp(ci)
                    if first and ci <= 13:
                        v_pt(0, ci + 2)
                    if ci == 0 and tail:
                        tail["attended"](CT - 2)
                    elif ci == 1 and tail:
                        tail["attended"](CT - 1)
                        emit_tail_attu2()
                    if ci >= 2:
                        attended(ci - 2)
                    if first:
                        if ci >= 14:
                            pop_filler(1)
                        continue
                    if ci == 3:
                        emit_tail_zb()
                    elif ci in (5, 7):
                        emit_tail_po((ci - 5) // 2)
                    elif ci in (2,):
                        pop_filler(1)
                    elif ci >= 8:
                        pop_filler(2)
                tail = {
                    "id": uid, "b": b, "qb": qb, "q0": q0,
                    "atts": atts, "attended": attended,
                }

            def emit_tail_attu2():
                # PSUM->SBUF copies for the (already fully accumulated) prev unit
                attus = []
                for h in range(2):
                    attu = zp.tile([65, QB], bf, tag="attu", name=f"au{tail['id']}{h}")
                    nc.vector.tensor_copy(attu[:], tail["atts"][h][:])
                    attus.append(attu)
                tail["attu"] = attus

            # ---- batch 0 projections (serial pre-phase) ------------------
            for ch in range(NQB):
                qk_chunk(ct_sb, wk_sb, bk_sb, kproj, 0, ch, "k")
            qk_chunk(qt_sb, wq_sb, bq_sb, qproj, 0, 0, "q")
            v_pt(0, 0)
            v_pt(0, 1)

            # ---- fillers (order respects downstream deadlines) -----------
            for ch in range(1, NQB):
                fillers.extend(qk_chunk_parts(qt_sb, wq_sb, bq_sb, qproj, 0, ch, "q"))
            for ch in range(NQB):
                fillers.extend(qk_chunk_parts(ct_sb, wk_sb, bk_sb, kproj, 1, ch, "k"))
            for pti in range(CT):
                fillers.extend(v_pt_parts(1, pti))
            for ch in range(NQB):
                fillers.extend(qk_chunk_parts(qt_sb, wq_sb, bq_sb, qproj, 1, ch, "q"))

            # ---- attention over all units --------------------------------
            for b in range(B):
                for qb in range(NQB):
                    attention_unit(b, qb)

            pop_filler(len(fillers))
            # fast final tail: finish last unit with scalar-engine copies
            tail["attended"](CT - 2)
            tail["attended"](CT - 1)
            attus = []
            for h in range(2):
                attu = zp.tile([65, QB], bf, tag="attu", name=f"auF{h}")
                nc.scalar.copy(attu[:], tail["atts"][h][:])
                attus.append(attu)
            tail["attu"] = attus
            emit_tail_zb()
            emit_tail_po(0)
            emit_tail_po(1)

    nc.compile()
    return nc


def get_nc():
    if "nc" not in _CACHE:
        _CACHE["nc"] = _build_nc()
    return _CACHE["nc"]


def make_in_maps(query, context, Wq, bq, Wk, bk, Wv, bv, Wo, bo):
    # qT/cT tile-contiguous: [ET, B, 128, QL]
    def pack_acts(x):
        xt = np.asarray(x, np.float32).reshape(POS, E).T.astype(BF16)  # [E, POS]
        xt = xt.reshape(ET, 128, B, QL).transpose(0, 2, 1, 3)  # [ET, B, 128, QL]
        return np.ascontiguousarray(xt).reshape(ET * B, 128, QL)

    qT = pack_acts(query)
    cT = pack_acts(context)

    def pack_w(Wslice):  # [E, P] -> [128, ET*P] e-tile-major
        w = np.asarray(Wslice, np.float32).reshape(ET, 128, P).transpose(1, 0, 2)
        return np.ascontiguousarray(w).astype(BF16).reshape(128, ET * P)

    in_maps = []
    for c in range(NCORES):
        F = slice(P * c, P * (c + 1))
        in_maps.append(
            {
                "qT": qT,
                "cT": cT,
                "wq": pack_w(Wq[:, F]),
                "wk": pack_w(Wk[:, F]),
                "wv": pack_w(Wv[:, F]),
                "wo": np.ascontiguousarray(Wo[F, :]).astype(BF16),
                "bq": np.ascontiguousarray(bq[F]).reshape(P, 1).astype(np.float32),
                "bk": np.ascontiguousarray(bk[F]).reshape(P, 1).astype(np.float32),
                "bvt": np.ascontiguousarray(
                    np.broadcast_to(bv[F], (128, P))
                ).astype(np.float32),
            }
        )
    return in_maps


def assemble_output(partials, bo):
    # partials: per-core [B*NQB*2, 128, OBW] tiles; value[p, ei*QB+q] of tile
    # (b, qb, g) = outT[(g*4+ei)*128 + p, b*QL + qb*QB + q]
    total = np.zeros((B * NQB * 2, 128, OBW), np.float32)
    for p in partials:
        total += p
    t = total.reshape(B, NQB, 2, 128, 4, QB).transpose(0, 1, 5, 2, 4, 3)
    out = t.reshape(B, QL, E) + np.asarray(bo, np.float32)
    return out.astype(np.float32)


def kernel(query, context, Wq, bq, Wk, bk, Wv, bv, Wo, bo):
    from concourse import bass_utils

    nc = get_nc()
    in_maps = make_in_maps(query, context, Wq, bq, Wk, bk, Wv, bv, Wo, bo)
    res = bass_utils.run_bass_kernel_spmd(nc, in_maps, core_ids=list(range(NCORES)))
    partials = [res.results[c]["outt"] for c in range(NCORES)]
    return assemble_output(partials, bo)
